# revision 1
# baseline (speedup 1.0000x reference)
"""Trainium2 Bass kernel for nn_ClassEmbedding: embedding gather + tanh
feeding a 2-layer LSTM (hidden 512, T=8) over a fused batch of 12800,
data-parallel over 8 NeuronCores (1600 rows/core).

Layout: everything transposed. Gates are computed as
    gatesT[4R, B] = W_ihT-contract(xT) + W_hhT-contract(hT)
so hidden states live as hT/cT [512 -> 4x128 chunks, B] and the recurrence
needs zero transposes. Only the 300-dim embeddings are transposed (PE
transpose, 128-token tiles).

Precision plan (temporal mixed precision): the LSTM forget gates sit near
0.5, so quantization error injected at step t decays ~0.5^(7-t) by the
final step. Steps 0..K8-1 therefore run with fp8e4m3 weights+activations
using DoubleRow matmuls (2 MACs/cell/cycle, K=256 per instruction); steps
K8..7 run the fp16 path. Weights for the fp8 steps are GPTQ-quantized on
the host against calibration activations from a small CPU reference run.
Scales: W x64, x/h x8 -> PSUM carries 512*gates, descaled for free via the
gate activation's scale=1/512.
"""
import sys

sys.path.insert(0, "/opt/trn_rl_repo")

import numpy as np
import ml_dtypes

from concourse import bass, mybir
import concourse.tile as tile
from concourse.bass_utils import run_bass_kernel_spmd
from concourse.masks import make_identity
from concourse.vector_clock import ScopedClock

F32 = mybir.dt.float32
F16 = mybir.dt.float16
F8 = mybir.dt.float8e4
I32 = mybir.dt.int32
AFT = mybir.ActivationFunctionType
DRM = mybir.MatmulPerfMode.DoubleRow
NP8 = ml_dtypes.float8_e4m3

P = 128
VOCAB, EMB, RNN, T = 20000, 300, 512, 8
B, NCLS = 64, 200
BN = B * NCLS            # 12800
NCORES = 8
BNC = BN // NCORES       # 1600 per core
PW = 400                 # pass width (batch columns per pass)
NPASS = BNC // PW        # 4
NM = 16                  # 2048 / 128 gate row chunks
EMBP = 384                                 # EMB zero-padded for K-chunking
EK = [(0, 128), (128, 128), (256, 44)]     # K-chunks of EMB=300 (data widths)
TOKT = [(0, 128), (128, 128), (256, 128), (384, 16)]  # token tiles per pass

K8 = 6                   # steps t < K8 run layer-2 in fp8; the rest fp16.
                         # Layer 1 runs fp8 at EVERY step: its quantization
                         # error is damped ~4x by layer-2's gate slope before
                         # reaching the output.
SW = 64.0                # fp8 weight scale
SA = 8.0                 # fp8 activation scale (products scaled SW*SA = 512)
DESC = 1.0 / (SW * SA)

GATE_BUFS = 14
TMP_BUFS = 6
EST_BUFS = 3


def _patched_drain_and_barrier(self, tick_clock, wait_clock):
    # walrus rejects >2 sync waits on one instruction; spread the final
    # drain's waits across single-wait NOPs.
    nc = self.nc
    drain_inst = nc.sync.drain()
    wait_clock.add_sem_waits(
        drain_inst.ins, ScopedClock({None: tick_clock.global_clock})
    )
    si = drain_inst.ins.sync_info
    if si is not None and si.on_wait and len(si.on_wait) > 1:
        waits = list(si.on_wait)
        si.on_wait = waits[:1]
        for w in waits[1:]:
            nop = nc.sync.nop()
            nop.ins.sync_info = mybir.SyncInfo(on_wait=[w], on_update=[])
    nc.all_engine_barrier()
    assert self.sems is not None
    popped = nc._tile_sem_poison_stack.pop()
    assert popped is self._sem_poison
    nc.clear_and_free_semaphores(list(self.sems.allocated().values()))
    nc.all_engine_barrier()


tile.TileContext._drain_and_barrier = _patched_drain_and_barrier


def _split_waits(nc, maxw=1):
    """walrus rejects instructions carrying more than a couple of sync
    waits; keep at most `maxw` on each instruction and move the rest to
    preceding same-engine NOPs."""
    wid = 0
    for bb in nc.main_func.blocks:
        out = []
        changed = False
        for inst in bb.instructions:
            si = inst.sync_info
            if si is not None and si.on_wait and len(si.on_wait) > maxw:
                waits = list(si.on_wait)
                for w in waits[maxw:]:
                    nop = mybir.InstNoOp(name=f"wsplit-{wid}", ins=[], outs=[])
                    wid += 1
                    nop.engine = inst.engine
                    nop.sync_info = mybir.SyncInfo(on_wait=[w], on_update=[])
                    out.append(nop)
                inst.sync_info = mybir.SyncInfo(
                    on_wait=waits[:maxw], on_update=list(si.on_update or [])
                )
                changed = True
            out.append(inst)
        if changed:
            bb.instructions = out


def build_nc():
    nc = bass.Bass()
    w2v16 = nc.declare_dram_parameter("w2v16", [VOCAB, EMB], F16, isOutput=False)
    wih2t = nc.declare_dram_parameter("wih2t", [RNN, 4 * RNN], F16, isOutput=False)
    whh2t = nc.declare_dram_parameter("whh2t", [RNN, 4 * RNN], F16, isOutput=False)
    q11d_d = nc.declare_dram_parameter("q11d", [P, 2, 4 * RNN], F8, isOutput=False)
    q11r_d = nc.declare_dram_parameter("q11r", [44, 4 * RNN], F8, isOutput=False)
    q1h_d = nc.declare_dram_parameter("q1h", [2, P, 2, 4 * RNN], F8, isOutput=False)
    q2i_d = nc.declare_dram_parameter("q2i", [2, P, 2, 4 * RNN], F8, isOutput=False)
    q2h_d = nc.declare_dram_parameter("q2h", [2, P, 2, 4 * RNN], F8, isOutput=False)
    b1d = nc.declare_dram_parameter("b1", [P, NM], F32, isOutput=False)
    b2d = nc.declare_dram_parameter("b2", [P, NM], F32, isOutput=False)
    idsd = nc.declare_dram_parameter("ids", [P, P], I32, isOutput=False)
    outd = nc.declare_dram_parameter("out", [RNN, BNC], F32, isOutput=True)

    with tile.TileContext(nc) as tc:
        with (
            tc.tile_pool(name="wp", bufs=1) as wp,
            tc.tile_pool(name="sp", bufs=1) as sp,
            tc.tile_pool(name="xp8", bufs=2) as xp8,
            tc.tile_pool(name="gb", bufs=GATE_BUFS) as gb,
            tc.tile_pool(name="tp", bufs=TMP_BUFS) as tp,
            tc.tile_pool(name="gp", bufs=5, space="PSUM") as gp,
            tc.tile_pool(name="tsp", bufs=3, space="PSUM") as tsp,
        ):
            # ---- small constants first: the sync DMA queue is FIFO, and
            # the gather pipeline only needs ids ----
            ids_sb = wp.tile([P, P], I32, name="ids_sb")
            nc.sync.dma_start(out=ids_sb[:], in_=idsd[:])
            b1_sb = wp.tile([P, NM], F32, name="b1_sb")
            nc.sync.dma_start(out=b1_sb[:], in_=b1d[:])
            b2_sb = wp.tile([P, NM], F32, name="b2_sb")
            nc.sync.dma_start(out=b2_sb[:], in_=b2d[:])
            ident32 = wp.tile([P, P], F32, name="ident32")
            make_identity(nc, ident32[:])
            ident = wp.tile([P, P], F16, name="ident")
            nc.vector.tensor_copy(out=ident[:], in_=ident32[:])

            # persistent embedding staging ring: 8 tiles = 2 timesteps of
            # gather prefetch so PE transposes never wait on the gather
            est_ring = [wp.tile([P, EMB], F16, name=f"est{i}") for i in range(8)]

            # ---- fp8 weights (needed first: step 0 is fp8) ----
            q11d = wp.tile([P, 2, 4 * RNN], F8, name="q11d")
            nc.sync.dma_start(out=q11d[:], in_=q11d_d[:])
            q11r = wp.tile([44, 4 * RNN], F8, name="q11r")
            nc.sync.dma_start(out=q11r[:], in_=q11r_d[:])

            def load_dr_w(dram, label):
                chunks = []
                for i in range(2):
                    wt = wp.tile([P, 2, 4 * RNN], F8, name=f"{label}_{i}")
                    nc.sync.dma_start(out=wt[:], in_=dram[i])
                    chunks.append(wt)
                return chunks

            q1h = load_dr_w(q1h_d, "q1h")
            q2i = load_dr_w(q2i_d, "q2i")
            q2h = load_dr_w(q2h_d, "q2h")

            # ---- fp16 weights (layer 2 late steps only) ----
            def load_rnn_w(dram, label):
                chunks = []
                for i in range(4):
                    wt = wp.tile([P, 4 * RNN], F16, name=f"{label}_{i}")
                    nc.sync.dma_start(out=wt[:], in_=dram[i * P : (i + 1) * P, :])
                    chunks.append(wt)
                return chunks

            w2i = load_rnn_w(wih2t, "w2i")
            w2h = load_rnn_w(whh2t, "w2h")

            # ---- persistent state tiles ----
            # fp8 DR layout: [ki, j, col] covers hidden rows 256*kb+128*j+ki
            h1d = [
                [sp.tile([P, 2, PW], F8, name=f"h1d_{bb}_{kb}") for kb in range(2)]
                for bb in range(2)
            ]
            h2d = [
                [sp.tile([P, 2, PW], F8, name=f"h2d_{bb}_{kb}") for kb in range(2)]
                for bb in range(2)
            ]
            h1 = [
                [sp.tile([P, PW], F16, name=f"h1_{bb}_{r}") for r in range(4)]
                for bb in range(2)
            ]
            h2 = [
                [sp.tile([P, PW], F16, name=f"h2_{bb}_{r}") for r in range(4)]
                for bb in range(2)
            ]
            h2f = [sp.tile([P, PW], F32, name=f"h2f_{r}") for r in range(4)]
            # c in fp16: 16-bit elementwise runs 2x on the vector engine;
            # the accuracy cost is ~1.2e-3 rel (simulated), inside budget
            c1 = [sp.tile([P, PW], F16, name=f"c1_{r}") for r in range(4)]
            c2 = [sp.tile([P, PW], F16, name=f"c2_{r}") for r in range(4)]

            def gen_gather(p_, t, slot):
                """Issue the 4 indirect gathers for (pass, step) into est
                ring slot (0/1)."""
                ests = []
                for j, (to, tn) in enumerate(TOKT):
                    g = (p_ * T + t) * len(TOKT) + j
                    est = est_ring[slot * 4 + j]
                    nc.gpsimd.indirect_dma_start(
                        out=est[:tn, :],
                        out_offset=None,
                        in_=w2v16[:],
                        in_offset=bass.IndirectOffsetOnAxis(
                            ap=ids_sb[:tn, g : g + 1], axis=0
                        ),
                    )
                    ests.append(est)
                return ests

            def gen_x(ests, t):
                """PE-transpose gathered [tokens, kchunk] tiles to
                [kchunk, tokens], moving PSUM->SBUF with the x8 scale and
                e4m3 conversion on the vector engine (layer 1 is fp8 at
                every step)."""
                xa = xp8.tile([P, 2, PW], F8, name="xa")
                xb = xp8.tile([P, PW], F8, name="xb")
                for j, (to, tn) in enumerate(TOKT):
                    est = ests[j]
                    for c, (ko, kw) in enumerate(EK):
                        tpp = tsp.tile([P, P], F16, name="tpp")
                        nc.tensor.transpose(
                            out=tpp[:kw, :tn],
                            in_=est[:tn, ko : ko + kw],
                            identity=ident[:tn, :tn],
                        )
                        dst = xa[:kw, c, to : to + tn] if c < 2 else \
                            xb[:kw, to : to + tn]
                        nc.vector.tensor_scalar_mul(dst, tpp[:kw, :tn], SA)
                return (xa, xb)

            def do_layer(ks, b_sb, t0flag, fp8):
                """ks: list of (lhsT_tile, kw_or_None, rhs_tile, is_dr)
                accumulated in order. Returns dict m -> gate tile [128, PW]
                (activated, bias added, descaled for fp8)."""
                ga = {}
                sc = DESC if fp8 else 1.0
                for r in range(4):
                    ms = [r, 8 + r, 12 + r] if t0flag else [r, 4 + r, 8 + r, 12 + r]
                    for mi in ms:
                        ps = gp.tile([P, PW], F32, name="ps")
                        nk = len(ks)
                        for kidx, (lt, kw, rt, is_dr) in enumerate(ks):
                            if is_dr:
                                nc.tensor.matmul(
                                    ps[:],
                                    lhsT=lt[:, :, mi * P : (mi + 1) * P],
                                    rhs=rt[:],
                                    start=(kidx == 0),
                                    stop=(kidx == nk - 1),
                                    perf_mode=DRM,
                                )
                            else:
                                nc.tensor.matmul(
                                    ps[:],
                                    lhsT=lt[:kw, mi * P : (mi + 1) * P],
                                    rhs=rt[:kw, :],
                                    start=(kidx == 0),
                                    stop=(kidx == nk - 1),
                                )
                        func = AFT.Tanh if mi // 4 == 2 else AFT.Sigmoid
                        g = gb.tile([P, PW], F16, name="gt")
                        nc.scalar.activation(
                            out=g[:], in_=ps[:], func=func,
                            bias=b_sb[:, mi : mi + 1], scale=sc,
                        )
                        ga[mi] = g
                return ga

            def update(ga, c, t0flag, out8=None, out16=None, outf=None):
                """c update + h writes. out8: fp8 DR tiles [2][128,2,PW]
                (written as SA*h), out16: fp16 chunk list, outf: fp32 chunk
                list. Any combination may be set."""
                for r in range(4):
                    i_, g_, o_ = ga[r], ga[8 + r], ga[12 + r]
                    if t0flag:
                        nc.vector.tensor_mul(out=c[r][:], in0=i_[:], in1=g_[:])
                    else:
                        f_ = ga[4 + r]
                        p1 = tp.tile([P, PW], F16, name="p1")
                        nc.vector.tensor_mul(out=p1[:], in0=f_[:], in1=c[r][:])
                        p2 = tp.tile([P, PW], F16, name="p2")
                        nc.vector.tensor_mul(out=p2[:], in0=i_[:], in1=g_[:])
                        nc.vector.tensor_add(out=c[r][:], in0=p1[:], in1=p2[:])
                    th = tp.tile([P, PW], F16, name="th")
                    nc.scalar.activation(out=th[:], in_=c[r][:], func=AFT.Tanh)
                    ndst = (out8 is not None) + (out16 is not None) + (outf is not None)
                    if ndst > 1:
                        hf = tp.tile([P, PW], F16, name="hf")
                        nc.vector.tensor_mul(out=hf[:], in0=o_[:], in1=th[:])
                        if out8 is not None:
                            kb, jj = r // 2, r % 2
                            nc.vector.tensor_scalar_mul(
                                out8[kb][:, jj, :], hf[:], SA
                            )
                        if out16 is not None:
                            nc.vector.tensor_copy(out=out16[r][:], in_=hf[:])
                        if outf is not None:
                            nc.vector.tensor_copy(out=outf[r][:], in_=hf[:])
                    elif out8 is not None:
                        kb, jj = r // 2, r % 2
                        hf = tp.tile([P, PW], F16, name="hf")
                        nc.vector.tensor_mul(out=hf[:], in0=o_[:], in1=th[:])
                        nc.vector.tensor_scalar_mul(out8[kb][:, jj, :], hf[:], SA)
                    elif out16 is not None:
                        nc.vector.tensor_mul(out=out16[r][:], in0=o_[:], in1=th[:])
                    else:
                        nc.vector.tensor_mul(out=outf[r][:], in0=o_[:], in1=th[:])

            NS = NPASS * T
            ests = {0: gen_gather(0, 0, 0), 1: gen_gather(0, 1, 1)}
            x_cur = gen_x(ests.pop(0), 0)
            for p_ in range(NPASS):
                for t in range(T):
                    s = p_ * T + t
                    wb = t % 2
                    rb = (t - 1) % 2
                    t0 = t == 0
                    fp8 = t < K8
                    # layer 1 (always fp8): x-part first, then hidden part
                    xa, xb = x_cur
                    ks1 = [(q11d, None, xa, True), (q11r, 44, xb, False)]
                    if not t0:
                        ks1 += [(q1h[kb], None, h1d[rb][kb], True)
                                for kb in range(2)]
                    g1 = do_layer(ks1, b1_sb, t0, True)
                    # h1 destinations: fp8 for the next step's L1 (and this
                    # step's L2 when fp8); fp16 when this step's L2 is fp16
                    o8 = h1d[wb] if t < T - 1 else None
                    o16 = h1[wb] if t >= K8 else None
                    update(g1, c1, t0, out8=o8, out16=o16)
                    # prefetch: gathers two steps ahead (gpsimd queue),
                    # transposes one step ahead (they fill the tensor-queue
                    # gap between layer-1 and layer-2 matmuls)
                    if s + 2 < NS:
                        np2, nt2 = divmod(s + 2, T)
                        ests[s + 2] = gen_gather(np2, nt2, (s + 2) % 2)
                    if s + 1 < NS:
                        x_next = gen_x(ests.pop(s + 1), (s + 1) % T)
                    else:
                        x_next = None
                    # layer 2: old-h2 hidden part first, h1-input last
                    if fp8:
                        ks2 = []
                        if not t0:
                            ks2 += [(q2h[kb], None, h2d[rb][kb], True)
                                    for kb in range(2)]
                        ks2 += [(q2i[kb], None, h1d[wb][kb], True)
                                for kb in range(2)]
                    else:
                        ks2 = []
                        if not t0:
                            ks2 += [(w2h[k], P, h2[rb][k], False) for k in range(4)]
                        ks2 += [(w2i[k], P, h1[wb][k], False) for k in range(4)]
                    g2 = do_layer(ks2, b2_sb, t0, fp8)
                    o8 = h2d[wb] if t + 1 < K8 else None
                    o16 = h2[wb] if K8 - 1 <= t < T - 1 else None
                    of = h2f if t == T - 1 else None
                    update(g2, c2, t0, out8=o8, out16=o16, outf=of)
                    x_cur = x_next
                # write this pass's final h2
                for r in range(4):
                    nc.sync.dma_start(
                        out=outd[r * P : (r + 1) * P, p_ * PW : (p_ + 1) * PW],
                        in_=h2f[r][:],
                    )
    _split_waits(nc)
    return nc


_NC_CACHE = None


def _get_nc():
    global _NC_CACHE
    if _NC_CACHE is None:
        _NC_CACHE = build_nc()
    return _NC_CACHE


def _sigmoid(x):
    return 1.0 / (1.0 + np.exp(-x))


def _gptq_e4m3(W, X, sc):
    """GPTQ-quantize W [M,K] to e4m3 at scale sc, calibrated on inputs
    X [N,K]. Returns the scaled quantized weights (fp32 values of sc*W)."""
    K = W.shape[1]
    H = (X.T @ X) / max(len(X), 1)
    H[np.diag_indices(K)] += 0.01 * np.mean(np.diag(H)) + 1e-8
    Hinv = np.linalg.inv(H)
    Wq = (W * sc).astype(np.float32).copy()
    Q = np.zeros_like(Wq)
    for j in range(K):
        q = Wq[:, j].astype(NP8).astype(np.float32)
        Q[:, j] = q
        err = (Wq[:, j] - q) / Hinv[j, j]
        if j + 1 < K:
            Wq[:, j + 1 :] -= np.outer(err, Hinv[j, j + 1 :])
    return Q


def _prep_core_inputs(sentence, word2vec, W_ih1, W_hh1, b_ih1, b_hh1,
                      W_ih2, W_hh2, b_ih2, b_hh2):
    f = lambda a: np.ascontiguousarray(np.asarray(a), dtype=np.float32)
    ids_all = np.asarray(sentence).reshape(BN, T).astype(np.int32)
    w2v = f(word2vec)
    w2v16 = np.tanh(w2v).astype(np.float16)
    W0 = {"W_ih1": f(W_ih1), "W_hh1": f(W_hh1),
          "W_ih2": f(W_ih2), "W_hh2": f(W_hh2)}
    b1 = f(b_ih1) + f(b_hh1)
    b2 = f(b_ih2) + f(b_hh2)

    # ---- fp16 weights (layer-2 late steps) ----
    fw = lambda a: np.ascontiguousarray(a.T.astype(np.float16))
    wih2t = fw(W0["W_ih2"])
    whh2t = fw(W0["W_hh2"])

    # ---- calibration run (CPU, fp32) for GPTQ Hessians ----
    CAL = 512
    cs = ids_all[:: max(BN // CAL, 1)][:CAL]
    h1 = np.zeros((CAL, RNN), np.float32)
    c1 = np.zeros_like(h1)
    h2 = np.zeros_like(h1)
    c2 = np.zeros_like(h1)
    Xx, Xh1, Xh1b, Xh2 = [], [], [], []
    for t in range(T):
        x = w2v16[cs[:, t]].astype(np.float32)
        Xx.append(x)
        Xh1.append(h1.copy())
        Xh2.append(h2.copy())
        g1 = x @ W0["W_ih1"].T + h1 @ W0["W_hh1"].T + b1
        i_, f_, g_, o_ = np.split(g1, 4, axis=1)
        c1 = _sigmoid(f_) * c1 + _sigmoid(i_) * np.tanh(g_)
        h1 = _sigmoid(o_) * np.tanh(c1)
        Xh1b.append(h1.copy())
        g2 = h1 @ W0["W_ih2"].T + h2 @ W0["W_hh2"].T + b2
        i_, f_, g_, o_ = np.split(g2, 4, axis=1)
        c2 = _sigmoid(f_) * c2 + _sigmoid(i_) * np.tanh(g_)
        h2 = _sigmoid(o_) * np.tanh(c2)

    Q = {}
    Q["W_ih1"] = _gptq_e4m3(W0["W_ih1"], np.concatenate(Xx) * SA, SW)
    Q["W_hh1"] = _gptq_e4m3(W0["W_hh1"], np.concatenate(Xh1) * SA, SW)
    Q["W_ih2"] = _gptq_e4m3(W0["W_ih2"], np.concatenate(Xh1b) * SA, SW)
    Q["W_hh2"] = _gptq_e4m3(W0["W_hh2"], np.concatenate(Xh2) * SA, SW)

    # ---- DR packs: tile[ki, j, m] = Q[m, 256*kb + 128*j + ki] ----
    def dr_pack(Qm, kb):
        lo = Qm[:, 256 * kb : 256 * kb + 128].T          # [128, M]
        hi = Qm[:, 256 * kb + 128 : 256 * kb + 256].T    # [128, M]
        return np.ascontiguousarray(
            np.stack([lo, hi], axis=1).astype(NP8))       # [128, 2, M]

    q11d = dr_pack(Q["W_ih1"], 0)
    q11r = np.ascontiguousarray(Q["W_ih1"][:, 256:300].T.astype(NP8))
    q1h = np.stack([dr_pack(Q["W_hh1"], kb) for kb in range(2)])
    q2i = np.stack([dr_pack(Q["W_ih2"], kb) for kb in range(2)])
    q2h = np.stack([dr_pack(Q["W_hh2"], kb) for kb in range(2)])

    b1m = f(b1.reshape(NM, P).T)
    b2m = f(b2.reshape(NM, P).T)

    in_maps = []
    for k in range(NCORES):
        ids_k = ids_all[k * BNC : (k + 1) * BNC]
        ids_arr = np.zeros((P, P), dtype=np.int32)
        for p_ in range(NPASS):
            for t in range(T):
                for j, (to, tn) in enumerate(TOKT):
                    g = (p_ * T + t) * len(TOKT) + j
                    ids_arr[:tn, g] = ids_k[p_ * PW + to : p_ * PW + to + tn, t]
        in_maps.append(
            {
                "w2v16": w2v16,
                "wih2t": wih2t,
                "whh2t": whh2t,
                "q11d": q11d,
                "q11r": q11r,
                "q1h": q1h,
                "q2i": q2i,
                "q2h": q2h,
                "b1": b1m,
                "b2": b2m,
                "ids": ids_arr,
            }
        )
    return in_maps


def kernel(sentence, word2vec, W_ih1, W_hh1, b_ih1, b_hh1,
           W_ih2, W_hh2, b_ih2, b_hh2, _trace=False, _return_perf=None):
    nc = _get_nc()
    in_maps = _prep_core_inputs(
        sentence, word2vec, W_ih1, W_hh1, b_ih1, b_hh1, W_ih2, W_hh2, b_ih2, b_hh2
    )
    res = run_bass_kernel_spmd(
        nc, in_maps, core_ids=list(range(NCORES)), trace=_trace
    )
    if _return_perf is not None:
        _return_perf.append(res)
    parts = [res.results[k]["out"].T for k in range(NCORES)]
    out = np.concatenate(parts, axis=0).reshape(B, NCLS, RNN)
    return np.ascontiguousarray(out, dtype=np.float32)

